# revision 22
# baseline (speedup 1.0000x reference)
"""Trainium2 Bass kernel for a 2-layer GCN (EnhancedGNN).

Computation (eval mode):
    src,dst,norm = gcn_norm(edge_index)            # sym deg^-1/2 with self loops
    h  = relu(gcn_layer(x, W1, b1))
    h  = gcn_layer(h, W2, b2)
    out = sigmoid(h @ Wl + bl)

Key identity: the per-edge norm dinv[src]*dinv[dst] factors into per-node
row scales, so  layer(X) = dinv * segsum(hs[src] -> dst) + b  with
hs = dinv * (X @ W) and the self loop as an ordinary edge.

Distribution: nodes sharded over 8 cores (6250 real + 22 fake zero rows
-> 6272 slots/core).  Edges live on the dst-owner core, sorted by dst.
Each 128-node output group is segment-summed on TensorE: gathered
message chunks [128 slots, 128 feat] (bf16) are multiplied by constant
0/1 selection matrices and accumulated in PSUM.  Chunks use a variable
slots-per-node d from a menu (node's segment fits one chunk), packed
greedily over the degree-sorted nodes; the chunk layout is built from
the elementwise-max degree profile across cores so all 8 cores run one
program.  Messages are fetched with batched dma_gather (int16 indices ->
two source banks of 25088 rows; a node's edges are processed in two
passes, one per bank).  The gather source (hs for all nodes, bf16) is
re-assembled each layer with an AllGather.
"""

import os
import sys

sys.path.insert(0, "/opt/trn_rl_repo")

import numpy as np

import concourse.bacc as bacc
import concourse.bass as bass
import concourse.tile as tile
from concourse import mybir
from concourse.bass_utils import run_bass_kernel_spmd

# ---------------------------------------------------------------- constants
N_REAL = 50000
E_EDGES = 800000
D = 128                      # feature dim
NC = 8                       # cores
SHARD_REAL = N_REAL // NC    # 6250
G = 49                       # node groups of 128 per core
SHARD = G * 128              # 6272 slots per core (incl 22 fakes)
NP = NC * SHARD              # 50176 padded node rows
HALF = NP // 2               # 25088 = bank size (< 32768 for int16 idx)
GCHUNK = 8                   # chunks (of 128 slots) per dma_gather call
                             # (1024 idxs max: >=1536 overflows the SWDGE ring)
NQ = 4                       # SWDGE queues to round-robin gathers over

# allowed slots-per-node values (chunk holds floor(128/d) nodes)
D_MENU = [1, 2, 3, 4, 5, 6, 7, 8, 10, 12, 14, 16, 18, 21, 25, 32, 42, 64, 128]

F32 = mybir.dt.float32
BF16 = mybir.dt.bfloat16
I16 = mybir.dt.int16


def _menu_ceil(x):
    for d in D_MENU:
        if d >= x:
            return d
    raise AssertionError(f"need {x} > 128 slots")


# ===================================================================== host
def _host_prep(x, edge_index):
    """Build per-core sharded inputs + the uniform static schedule."""
    src = np.asarray(edge_index[0], dtype=np.int64)
    dst = np.asarray(edge_index[1], dtype=np.int64)

    deg = np.bincount(dst, minlength=N_REAL).astype(np.int64) + 1  # + self loop

    order = np.argsort(dst, kind="stable")
    s_src = src[order]
    s_dst = dst[order]
    starts = np.searchsorted(s_dst, np.arange(N_REAL), side="left")
    ends = np.searchsorted(s_dst, np.arange(N_REAL), side="right")

    src_bank = (s_src >= (SHARD_REAL * 4)).astype(np.int8)
    own_bank = (np.arange(N_REAL) >= (SHARD_REAL * 4)).astype(np.int64)

    n_in = np.zeros((N_REAL, 2), dtype=np.int64)
    np.add.at(n_in, (s_dst, src_bank.astype(np.int64)), 1)
    n_in[np.arange(N_REAL), own_bank] += 1  # self loop

    # ---- per-core permutation pi: sort slots by (need0, need1); fakes first
    pis = []
    needs = []          # per core [SHARD, 2] in pi order
    rows_of_real = np.full(N_REAL, -1, dtype=np.int64)
    for c in range(NC):
        lo = c * SHARD_REAL
        need = np.ones((SHARD, 2), dtype=np.int64)
        need[:SHARD_REAL] = np.maximum(n_in[lo:lo + SHARD_REAL], 1)
        key = need[:, 0] * 256 + need[:, 1]
        pi = np.argsort(key, kind="stable")
        local = np.where(pi < SHARD_REAL, pi, -1)
        pis.append(local)
        needs.append(need[pi])
        mask = local >= 0
        rows_of_real[lo + local[mask]] = c * SHARD + np.nonzero(mask)[0]
    assert (rows_of_real >= 0).all()

    fake_rows = []
    for bank, c in ((0, 0), (1, 4)):
        fslots = np.nonzero(pis[c] < 0)[0]
        assert len(fslots) > 0
        fake_rows.append(c * SHARD + fslots[0])
    assert fake_rows[0] < HALF <= fake_rows[1]

    # ---- uniform max profile across cores, then chunk layouts
    max_need = np.maximum.reduce(needs)            # [SHARD, 2]

    # chunk layout per (g, p): list of (d, base, n_nodes); greedy over the
    # sorted max-need profile.  All cores share this layout.
    layouts = {}
    sched = []        # (g, p, chunks, idx_off) in program order
    off = 0
    for g in range(G):
        for p in range(2):
            prof = max_need[g * 128:(g + 1) * 128, p]
            chunks = []
            pos = 0
            while pos < 128:
                d = _menu_ceil(prof[pos])
                cap = 128 // d
                take = 1
                while (take < cap and pos + take < 128
                       and prof[pos + take] <= d):
                    take += 1
                # grow d if the next node's need is only slightly above and
                # merging saves a chunk -- skip; greedy is close to optimal
                chunks.append((d, pos, take))
                pos += take
            layouts[(g, p)] = chunks
            sched.append((g, p, chunks, off))
            off += 128 * len(chunks)
    tot_slots = off

    # ---- per-core gather indices
    idx_maps = []
    xT_maps = []
    deg_maps = []
    for c in range(NC):
        lo = c * SHARD_REAL
        idx_flat = np.empty(tot_slots, dtype=np.int16)
        for (g, p, chunks, o) in sched:
            for ci, (d, base, take) in enumerate(chunks):
                blk = np.full((128,), fake_rows[p] - p * HALF, dtype=np.int64)
                for t in range(take):
                    slot = g * 128 + base + t
                    lreal = pis[c][slot]
                    if lreal < 0:
                        continue
                    v = lo + lreal
                    e0, e1 = starts[v], ends[v]
                    bsel = src_bank[e0:e1] == p
                    rows = rows_of_real[s_src[e0:e1][bsel]]
                    if own_bank[v] == p:
                        rows = np.concatenate([rows, [rows_of_real[v]]])
                    assert len(rows) <= d, (len(rows), d)
                    blk[t * d:t * d + len(rows)] = rows - p * HALF
                idx_flat[o + ci * 128:o + (ci + 1) * 128] = blk.astype(np.int16)
        wrapped = idx_flat.reshape(-1, 16).T.copy()
        idx_maps.append(np.tile(wrapped, (8, 1)))        # [128, tot/16]

        xT = np.zeros((D, SHARD), dtype=np.float32)
        mask = pis[c] >= 0
        xT[:, mask] = np.asarray(x)[lo + pis[c][mask]].T
        xT_maps.append(np.ascontiguousarray(xT))

        dg = np.ones(SHARD, dtype=np.float32)
        dg[mask] = deg[lo + pis[c][mask]].astype(np.float32)
        deg_maps.append(np.ascontiguousarray(dg.reshape(G, 128).T))

    # ---- selection matrices, one per distinct d.  Chunk at psum base b
    # uses slice [:, 127-b : 255-b]; ones sit at [s, 127 + s//d], s < m*d.
    d_set = sorted({d for chunks in layouts.values() for (d, _, _) in chunks})
    w_ext = {}
    for d in d_set:
        m = 128 // d
        w = np.zeros((128, 255), dtype=np.float32)
        s = np.arange(m * d)
        w[s, 127 + s // d] = 1.0
        w_ext[d] = w

    return dict(
        sched=sched, tot_slots=tot_slots, d_set=d_set, w_ext=w_ext,
        idx_maps=idx_maps, xT_maps=xT_maps, deg_maps=deg_maps,
        pis=pis, rows_of_real=rows_of_real, deg=deg,
    )


# ==================================================================== device
def _build_nc(prep, has_b1, has_b2):
    sched = prep["sched"]
    d_set = prep["d_set"]
    tot_slots = prep["tot_slots"]

    nc = bacc.Bacc("TRN2", target_bir_lowering=False, num_devices=NC,
                   num_swdge_queues=NQ)
    core_ids = list(range(NC))

    # ---- I/O
    xT_in = nc.declare_dram_parameter("xT", [D, SHARD], F32, isOutput=False)
    degg_in = nc.declare_dram_parameter("deg_g", [128, G], F32, isOutput=False)
    idx_in = nc.declare_dram_parameter(
        "idx_all", [128, tot_slots // 16], I16, isOutput=False)
    w1_in = nc.declare_dram_parameter("W1", [D, D], F32, isOutput=False)
    w2_in = nc.declare_dram_parameter("W2", [D, D], F32, isOutput=False)
    wlb_in = nc.declare_dram_parameter("Wl_bcast", [128, D], F32, isOutput=False)
    blr_in = nc.declare_dram_parameter("bl_rep", [128, 1], F32, isOutput=False)
    b1b_in = nc.declare_dram_parameter("b1_bcast", [128, D], F32, isOutput=False)
    b2b_in = nc.declare_dram_parameter("b2_bcast", [128, D], F32, isOutput=False)
    wexts_in = {
        d: nc.declare_dram_parameter(
            f"w_ext_{d}", [128, 255], BF16, isOutput=False)
        for d in d_set
    }
    ident_in = nc.declare_dram_parameter("ident", [128, 128], F32, isOutput=False)
    out_ext = nc.declare_dram_parameter("out", [SHARD, 1], F32, isOutput=True)

    # ---- internal DRAM (gather sources in bf16)
    hs1_shard = nc.dram_tensor("hs1_shard", [SHARD, D], BF16)
    hs2_shard = nc.dram_tensor("hs2_shard", [SHARD, D], BF16)
    hs1_ag = nc.dram_tensor("hs1_ag", [NP, D], BF16, addr_space="Shared")
    hs2_ag = nc.dram_tensor("hs2_ag", [NP, D], BF16, addr_space="Shared")

    from contextlib import ExitStack
    with tile.TileContext(nc) as tc, ExitStack() as es:
        cpool = es.enter_context(tc.tile_pool(name="const", bufs=1))
        gpool = es.enter_context(tc.tile_pool(name="gather", bufs=6))
        spool = es.enter_context(tc.tile_pool(name="stage", bufs=4))
        ppool = es.enter_context(tc.tile_pool(name="psum", bufs=4, space="PSUM"))
        ppool2 = es.enter_context(tc.tile_pool(name="psum2", bufs=2, space="PSUM"))

        # ---------------- persistent SBUF
        xT_t = cpool.tile([D, SHARD], F32, tag="xT")
        nc.sync.dma_start(out=xT_t[:], in_=xT_in[:])
        w1_t = cpool.tile([D, D], F32, tag="w1")
        nc.sync.dma_start(out=w1_t[:], in_=w1_in[:])
        w2_t = cpool.tile([D, D], F32, tag="w2")
        nc.sync.dma_start(out=w2_t[:], in_=w2_in[:])
        wlb_t = cpool.tile([128, D], F32, tag="wlb")
        nc.sync.dma_start(out=wlb_t[:], in_=wlb_in[:])
        blr_t = cpool.tile([128, 1], F32, tag="blr")
        nc.sync.dma_start(out=blr_t[:], in_=blr_in[:])
        b1b_t = cpool.tile([128, D], F32, tag="b1b")
        nc.sync.dma_start(out=b1b_t[:], in_=b1b_in[:])
        b2b_t = cpool.tile([128, D], F32, tag="b2b")
        nc.sync.dma_start(out=b2b_t[:], in_=b2b_in[:])
        idx_t = cpool.tile([128, tot_slots // 16], I16, tag="idx")
        nc.sync.dma_start(out=idx_t[:], in_=idx_in[:])
        wext_t = {}
        for d in d_set:
            t = cpool.tile([128, 255], BF16, tag=f"wext{d}")
            nc.sync.dma_start(out=t[:], in_=wexts_in[d][:])
            wext_t[d] = t

        degg_t = cpool.tile([128, G], F32, tag="degg")
        nc.sync.dma_start(out=degg_t[:], in_=degg_in[:])
        sdeg_t = cpool.tile([128, G], F32, tag="sdeg")
        nc.scalar.sqrt(sdeg_t[:], degg_t[:])
        dinv_t = cpool.tile([128, G], F32, tag="dinv")
        nc.vector.reciprocal(dinv_t[:], sdeg_t[:])

        ident_t = cpool.tile([128, 128], F32, tag="ident")
        nc.sync.dma_start(out=ident_t[:], in_=ident_in[:])

        h1s_all = cpool.tile([128, G * D], F32, tag="h1s")
        h2_all = cpool.tile([128, G * D], F32, tag="h2")

        # collapse const-load DMA sems so early matmuls stay 1-wait
        tc.strict_bb_all_engine_barrier()

        # ---------------- phase B: hs1 = bf16(dinv * (x @ W1)), shard rows
        for g in range(G):
            ps = ppool2.tile([128, D], F32, space="PSUM", tag="mmps")
            nc.tensor.matmul(ps[:], lhsT=xT_t[:, g * 128:(g + 1) * 128],
                             rhs=w1_t[:], start=True, stop=True)
            st = spool.tile([128, D], BF16, tag="bstage")
            nc.scalar.activation(st[:], ps[:], mybir.ActivationFunctionType.Copy,
                                 bias=0.0, scale=dinv_t[:, g:g + 1])
            nc.sync.dma_start(out=hs1_shard[g * 128:(g + 1) * 128, :], in_=st[:])

        nc.gpsimd.collective_compute(
            "AllGather", mybir.AluOpType.bypass,
            replica_groups=[core_ids],
            ins=[hs1_shard[:]], outs=[hs1_ag[:]],
        )

        qctr = [0]

        # ---------------- message passing (shared by both layers)
        def message_pass(src_ag, out_all, relu, extra_dinv, bias_t, has_b):
            banks = [src_ag[0:HALF, :], src_ag[HALF:NP, :]]
            for g in range(G):
                ps = ppool.tile([128, D], F32, space="PSUM", tag="segps")
                entries = [e for e in sched if e[0] == g]
                total_chunks = sum(len(e[2]) for e in entries)
                done = 0
                first = True
                for (gg, p, chunks, o) in entries:
                    n_sub = (len(chunks) + GCHUNK - 1) // GCHUNK
                    for sub in range(n_sub):
                        c_lo = sub * GCHUNK
                        c_hi = min(c_lo + GCHUNK, len(chunks))
                        ncnk = c_hi - c_lo
                        gt = gpool.tile([128, GCHUNK * D], BF16, tag="gmsg")
                        n_idx = ncnk * 128
                        q = qctr[0] % NQ
                        qctr[0] += 1
                        nc.gpsimd.dma_gather(
                            gt[:, :ncnk * D].rearrange("p (c f) -> p c f", f=D),
                            banks[p],
                            idx_t[:, (o + c_lo * 128) // 16:(o + c_hi * 128) // 16],
                            n_idx, n_idx, D, queue_num=q,
                        )
                        for ci in range(ncnk):
                            d, base, take = chunks[c_lo + ci]
                            nc.tensor.matmul(
                                ps[:],
                                lhsT=wext_t[d][:, 127 - base:255 - base],
                                rhs=gt[:, ci * D:(ci + 1) * D],
                                start=first, stop=(done + 1 == total_chunks),
                            )
                            first = False
                            done += 1
                # epilogue: out = [relu](dinv * seg + b) [* dinv]
                dv = dinv_t[:, g:g + 1]
                dst = out_all[:, g * D:(g + 1) * D]
                A = mybir.ActivationFunctionType
                if has_b:
                    t1 = spool.tile([128, D], F32, tag="ep1")
                    nc.scalar.activation(t1[:], ps[:], A.Copy, bias=0.0, scale=dv)
                    t2 = spool.tile([128, D], F32, tag="ep2")
                    nc.vector.tensor_add(t2[:], t1[:], bias_t[:])
                    if relu:
                        t3 = spool.tile([128, D], F32, tag="ep3")
                        nc.scalar.activation(t3[:], t2[:], A.Relu)
                        src_t = t3
                    else:
                        src_t = t2
                    if extra_dinv:
                        nc.scalar.activation(dst, src_t[:], A.Copy,
                                             bias=0.0, scale=dv)
                    else:
                        nc.vector.tensor_copy(dst, src_t[:])
                else:
                    if relu and extra_dinv:
                        t1 = spool.tile([128, D], F32, tag="ep1")
                        nc.scalar.activation(t1[:], ps[:], A.Relu,
                                             bias=0.0, scale=dv)
                        nc.scalar.activation(dst, t1[:], A.Copy,
                                             bias=0.0, scale=dv)
                    elif relu:
                        nc.scalar.activation(dst, ps[:], A.Relu,
                                             bias=0.0, scale=dv)
                    else:
                        nc.scalar.activation(dst, ps[:], A.Copy,
                                             bias=0.0, scale=dv)

        phases = os.environ.get("GNN_PHASES", "all")

        def debug_out(src_tile, col):
            for g in range(G):
                st = spool.tile([128, 1], F32, tag="fout")
                nc.vector.tensor_copy(st[:], src_tile[:, g * col:g * col + 1])
                nc.sync.dma_start(out=out_ext[g * 128:(g + 1) * 128, :], in_=st[:])

        if phases == "B":
            debug_out(dinv_t, 1)
        if phases not in ("B",):
            # layer 1: H1s = dinv * relu(dinv*seg + b1)
            message_pass(hs1_ag, h1s_all, relu=True, extra_dinv=True,
                         bias_t=b1b_t, has_b=has_b1)
            if phases == "BC":
                debug_out(h1s_all, D)

        if phases not in ("B", "BC"):
            # ------------ phase D: hs2 = bf16(H1s @ W2) (shard) + AllGather
            for g in range(G):
                pt = ppool2.tile([128, D], F32, space="PSUM", tag="tps")
                nc.tensor.transpose(pt[:], h1s_all[:, g * D:(g + 1) * D],
                                    ident_t[:])
                tt = spool.tile([128, D], F32, tag="ttile")
                nc.vector.tensor_copy(tt[:], pt[:])
                ps = ppool2.tile([128, D], F32, space="PSUM", tag="mmps")
                nc.tensor.matmul(ps[:], lhsT=tt[:], rhs=w2_t[:],
                                 start=True, stop=True)
                st = spool.tile([128, D], BF16, tag="bstage")
                nc.vector.tensor_copy(st[:], ps[:])
                nc.sync.dma_start(out=hs2_shard[g * 128:(g + 1) * 128, :],
                                  in_=st[:])

            nc.gpsimd.collective_compute(
                "AllGather", mybir.AluOpType.bypass,
                replica_groups=[core_ids],
                ins=[hs2_shard[:]], outs=[hs2_ag[:]],
            )

            # ------------ phase E: layer-2 message passing (no relu)
            message_pass(hs2_ag, h2_all, relu=False, extra_dinv=False,
                         bias_t=b2b_t, has_b=has_b2)

            # ------------ phase F: out = sigmoid(H2 @ Wl + bl)
            for g in range(G):
                mt = spool.tile([128, D], F32, tag="fmul")
                nc.vector.tensor_tensor(out=mt[:],
                                        in0=h2_all[:, g * D:(g + 1) * D],
                                        in1=wlb_t[:], op=mybir.AluOpType.mult)
                rt = spool.tile([128, 1], F32, tag="fred")
                nc.vector.tensor_reduce(rt[:], mt[:], axis=mybir.AxisListType.X,
                                        op=mybir.AluOpType.add)
                ot = spool.tile([128, 1], F32, tag="fout")
                nc.scalar.activation(ot[:], rt[:],
                                     mybir.ActivationFunctionType.Sigmoid,
                                     bias=blr_t[:], scale=1.0)
                nc.sync.dma_start(out=out_ext[g * 128:(g + 1) * 128, :], in_=ot[:])

    nc.compile()
    return nc


# ==================================================================== entry
_CACHE = {}


def kernel(x, edge_index, W1, b1, W2, b2, Wl, bl):
    import ml_dtypes  # noqa: F401  (registers bfloat16 with numpy)

    x = np.asarray(x, dtype=np.float32)
    edge_index = np.asarray(edge_index)
    W1 = np.asarray(W1, dtype=np.float32)
    W2 = np.asarray(W2, dtype=np.float32)
    Wl = np.asarray(Wl, dtype=np.float32)
    b1 = np.asarray(b1, dtype=np.float32)
    b2 = np.asarray(b2, dtype=np.float32)
    bl = np.asarray(bl, dtype=np.float32)

    prep = _host_prep(x, edge_index)
    has_b1 = bool(np.any(b1))
    has_b2 = bool(np.any(b2))

    nc = _build_nc(prep, has_b1, has_b2)

    wl_bcast = np.tile(Wl.reshape(1, D), (128, 1)).astype(np.float32)
    bl_rep = np.full((128, 1), float(bl.reshape(-1)[0]), dtype=np.float32)
    b1_bcast = np.tile(b1.reshape(1, D), (128, 1)).astype(np.float32)
    b2_bcast = np.tile(b2.reshape(1, D), (128, 1)).astype(np.float32)

    import ml_dtypes as mld
    in_maps = []
    for c in range(NC):
        m = {
            "xT": prep["xT_maps"][c],
            "deg_g": prep["deg_maps"][c],
            "idx_all": prep["idx_maps"][c],
            "W1": W1, "W2": W2,
            "Wl_bcast": wl_bcast, "bl_rep": bl_rep,
            "b1_bcast": b1_bcast, "b2_bcast": b2_bcast,
        }
        for d, w in prep["w_ext"].items():
            m[f"w_ext_{d}"] = np.asarray(w, dtype=mld.bfloat16)
        m["ident"] = np.eye(128, dtype=np.float32)
        in_maps.append(m)

    trace = bool(os.environ.get("GNN_TRACE"))
    kw = {}
    if trace:
        kw = dict(trace=True, tmpdir=os.environ.get("GNN_TRACE_DIR") or None)
    res = run_bass_kernel_spmd(nc, in_maps, list(range(NC)), **kw)
    _CACHE["last_result"] = res

    out = np.empty((N_REAL, 1), dtype=np.float32)
    for c in range(NC):
        o = res.results[c]["out"]          # [SHARD, 1], pi order
        pi = prep["pis"][c]
        mask = pi >= 0
        out[c * SHARD_REAL + pi[mask], 0] = o[mask, 0]
    return out


if __name__ == "__main__":
    rng = np.random.default_rng(0)
    x = rng.standard_normal((N_REAL, D), dtype=np.float32)
    ei = rng.integers(0, N_REAL, size=(2, E_EDGES), dtype=np.int64)
    W1 = rng.standard_normal((D, D), dtype=np.float32) / np.sqrt(D)
    W2 = rng.standard_normal((D, D), dtype=np.float32) / np.sqrt(D)
    Wl = rng.standard_normal((D, 1), dtype=np.float32) / np.sqrt(D)
    z = np.zeros(D, dtype=np.float32)
    out = kernel(x=x, edge_index=ei, W1=W1, b1=z, W2=W2, b2=z,
                 Wl=Wl, bl=np.zeros(1, dtype=np.float32))
    print(out.shape, out[:5, 0])
